# revision 14
# baseline (speedup 1.0000x reference)
"""DDFConvGuidedFilter Trainium2 kernel.

Data-parallel over batch: 16 images -> 8 cores x 2 images.

Per image (512x512, C=3), three guided-filter stages (k=3,7,15; eps=.16,.04,.01):
  s = sum_c x_c ; q = sum_c x_c^2
  per stage:  mean = box(s)/Nc ; Q = box(q)/Nc        (Nc = 3*cnt_h*cnt_w)
              var = Q - mean^2 ; r = 1/(var+eps)
              A3 = 3 - 3*eps*r (= 3A);  rm3 = 3*eps*r*mean (= 3b)
              mA = box(A3) ; mB = box(rm3)            (= C*box(A), C*box(b))
              next: s' = mA*s + 3mB ; q' = mA^2 q + 2 mA mB s + 3 mB^2
  F_j = G_j*x + H_j with G1=mA1, H1=mB1, G2=mA2*G1, H2=mA2*H1+mB2, ...
  out_o = sum_jc w1[o,3(j-1)+c] * (u_j * x_c)  + beta_o2*v2 + beta_o3*v3 - beta_o1*H1
    u1=1-G1, u2=G1-G2, u3=(1-mA3)*G2, v2=H1-H2, v3=(1-mA3)*H2-mB3

Box filters: horizontal pass = ONE flat DVE tensor_tensor_scan per map (the
zero margins between row-blocks self-correct the running window); vertical
pass = PE matmul against banded 128x128 Toeplitz blocks (with 1/Nc
row-normalization folded into the s/q bands; w-edge correction applied to 7
edge columns afterwards).  Final 1x1 conv = DVE scalar chains (per-partition
scalar pointers carry the w1 weights), not PE.
"""

import numpy as np

H = W = 512
C = 3
NB = 4  # h blocks of 128
OFF = 16  # left zero margin in padded tiles
BW = OFF + W + 8  # padded tile width = 536
STAGES = [(3, 0.16), (7, 0.04), (15, 0.01)]
N_CORES = 8
IMGS_PER_CORE = 2
NWS = 40  # w-scalar slots


def _cnt(k, n=512):
    i = np.arange(n)
    p = (k - 1) // 2
    return np.minimum(i + p + 1, np.minimum(2 * p + 1, n - i + p)).astype(np.float64)


def _band_block(k, dj, row_scale=None):
    """lhsT[kk, m] = scale(h_out) * 1(|128*dj + kk - m| <= pad)."""
    pad = (k - 1) // 2
    m = np.arange(128)
    kk = np.arange(128)[:, None]
    mat = (np.abs(128 * dj + kk - m) <= pad).astype(np.float64)
    if row_scale is not None:
        mat = mat * row_scale[None, :]
    return mat


def _make_consts():
    """Band matrices [24,128,128] and edge-gamma tiles [3,2,128,4,7]."""
    bands = []
    gedges = np.zeros((3, 2, 128, NB, 7), np.float64)
    for si, (k, eps) in enumerate(STAGES):
        ch = _cnt(k)
        alpha = 1.0 / (3.0 * ch * k)  # row scale: 1/(3*cnt_h(h)*k)
        # sq bands: diag0, diagM, diag3, upM, dnM
        bands.append(_band_block(k, 0, alpha[0:128]))
        bands.append(_band_block(k, 0, alpha[128:256]))
        bands.append(_band_block(k, 0, alpha[384:512]))
        bands.append(_band_block(k, 1, alpha[128:256]))  # interior rows
        bands.append(_band_block(k, -1, alpha[128:256]))
        # A/rm bands: unnormalized
        bands.append(_band_block(k, 0))
        bands.append(_band_block(k, 1))
        bands.append(_band_block(k, -1))
        cw = _cnt(k)
        gl = k / cw[0:7]
        gr = k / cw[505:512]
        gedges[si, 0] = np.broadcast_to(gl[None, None, :], (128, NB, 7))
        gedges[si, 1] = np.broadcast_to(gr[None, None, :], (128, NB, 7))
    import ml_dtypes
    return np.stack(bands).astype(ml_dtypes.bfloat16), gedges.astype(np.float32)


# band index helpers: per stage si, base = si*8
SQ_DIAG = {0: 0, 1: 1, 2: 1, 3: 2}  # per out-block
SQ_UP, SQ_DN = 3, 4
A_DIAG, A_UP, A_DN = 5, 6, 7


def _make_wscalars(w1):
    """[128, NWS] f32: w1[o,i] at o*9+i; -beta1[o], beta2[o], beta3[o] at 27+3o+j."""
    vals = np.zeros(NWS, np.float64)
    for o in range(3):
        for i in range(9):
            vals[o * 9 + i] = w1[o, i]
    beta = w1.reshape(3, 3, 3).sum(axis=2)  # [o, j]
    for o in range(3):
        vals[27 + 3 * o + 0] = -beta[o, 0]
        vals[27 + 3 * o + 1] = beta[o, 1]
        vals[27 + 3 * o + 2] = beta[o, 2]
    return np.broadcast_to(vals[None, :], (128, NWS)).astype(np.float32).copy()


def _build_program(reps=1, w1=None):
    import concourse.bacc as bacc
    import concourse.tile as tile
    from concourse import mybir

    assert w1 is not None, "w1 weights are baked as immediates"
    w1f = np.asarray(w1, np.float64)
    beta = w1f.reshape(3, 3, 3).sum(axis=2)  # [o, j]

    F32 = mybir.dt.float32
    BF16 = mybir.dt.bfloat16
    ADD = mybir.AluOpType.add
    SUB = mybir.AluOpType.subtract
    MUL = mybir.AluOpType.mult
    SQUARE = mybir.ActivationFunctionType.Square
    COPY = mybir.ActivationFunctionType.Copy

    nc = bacc.Bacc("TRN2", target_bir_lowering=False, debug=False,
                   num_devices=N_CORES)
    xs_ap = nc.dram_tensor("xs", [IMGS_PER_CORE, C, H, W], BF16,
                           kind="ExternalInput").ap()
    bands_ap = nc.dram_tensor("bands", [24, 128, 128], BF16,
                              kind="ExternalInput").ap()
    gedge_ap = nc.dram_tensor("gedge", [3, 2, 128, NB, 7], F32,
                              kind="ExternalInput").ap()
    wsc_ap = nc.dram_tensor("wsc", [128, NWS], F32, kind="ExternalInput").ap()
    out_ap = nc.dram_tensor("out", [IMGS_PER_CORE, C, H, W], BF16,
                            kind="ExternalOutput").ap()

    with tile.TileContext(nc) as tc:
        with (
            tc.tile_pool(name="const", bufs=1) as constp,
            tc.tile_pool(name="fixed", bufs=1) as fixed,
            tc.tile_pool(name="scan", bufs=1) as scanp,
            tc.tile_pool(name="scr", bufs=4) as scr,
            tc.tile_pool(name="pers", bufs=1) as pers,
            tc.tile_pool(name="pp", bufs=2) as pp,
            tc.tile_pool(name="tp", bufs=2) as tp,
            tc.tile_pool(name="obp", bufs=1) as obp,
            tc.tile_pool(name="psum", bufs=2, space="PSUM") as psum,
        ):
            # ---- constants ----
            bands = constp.tile([128, 24, 128], BF16)
            nc.sync.dma_start(bands[:], bands_ap.rearrange("n k m -> k n m"))
            gedge = constp.tile([128, 3, 2, NB, 7], F32)
            nc.sync.dma_start(gedge[:], gedge_ap.rearrange("s e p b j -> p s e b j"))
            wsc = constp.tile([128, NWS], F32)
            nc.sync.dma_start(wsc[:], wsc_ap)

            # ---- fixed padded map tiles (shared across stages & images) ----
            s_t = fixed.tile([128, NB, BW], F32, tag="s_t")
            q_t = fixed.tile([128, NB, BW], F32, tag="q_t")
            a_t = fixed.tile([128, NB, BW], F32, tag="a_t")
            rm_t = fixed.tile([128, NB, BW], F32, tag="rm_t")
            for t in (s_t, q_t, a_t, rm_t):
                nc.vector.memset(t[:], 0.0)

            # fused bf16 persists: [128, img, NB, W]
            G1 = pers.tile([128, IMGS_PER_CORE, NB, W], BF16, tag="G1")
            H1 = pers.tile([128, IMGS_PER_CORE, NB, W], BF16, tag="H1")
            G2 = pers.tile([128, IMGS_PER_CORE, NB, W], BF16, tag="G2")
            Hh2 = pers.tile([128, IMGS_PER_CORE, NB, W], BF16, tag="H2")
            u1 = pers.tile([128, IMGS_PER_CORE, NB, W], BF16, tag="u1")
            u2 = pers.tile([128, IMGS_PER_CORE, NB, W], BF16, tag="u2")
            u3, v3 = G1, G2  # slots reused after G1/G2 die
            x_t = pers.tile([128, IMGS_PER_CORE, C, NB, W], BF16, tag="x")

            def ctr(t):  # central (data) region of a padded tile
                return t[:, :, OFF:OFF + W]

            def hscan(padded, k):
                """H box pass: ONE flat scan.  hs center = [:, b, OFF+pad : OFF+pad+W]."""
                pad = (k - 1) // 2
                hs = scanp.tile([128, NB, BW], BF16, tag="hs")
                flat_in = padded[:].rearrange("p b w -> p (b w)")
                flat_out = hs[:].rearrange("p b w -> p (b w)")
                TOT = NB * BW
                nc.vector.tensor_tensor_scan(
                    flat_out[:, k:TOT],
                    flat_in[:, k:TOT],
                    flat_in[:, 0:TOT - k],
                    0.0, ADD, SUB,
                )
                return hs, pad

            def vband(hs, pad, si, sq, tag):
                """V box pass on PE -> psum tile [128, NB, 512]."""
                ps = psum.tile([128, NB, W], F32, tag="ps")
                base = si * 8
                o0 = OFF + pad
                for b in range(NB):
                    js = [j for j in (b - 1, b, b + 1) if 0 <= j < NB]
                    for idx, j in enumerate(js):
                        if sq:
                            bi = base + (SQ_DIAG[b] if j == b else
                                         (SQ_UP if j == b + 1 else SQ_DN))
                        else:
                            bi = base + (A_DIAG if j == b else
                                         (A_UP if j == b + 1 else A_DN))
                        nc.tensor.matmul(
                            ps[:, b, :], bands[:, bi, :],
                            hs[:, j, o0:o0 + W],
                            start=(idx == 0), stop=(idx == len(js) - 1),
                        )
                return ps

            def edgefix(ps, si):
                """multiply 7 left/right edge columns by gamma (in-place, PSUM)."""
                nc.vector.tensor_tensor(
                    ps[:, :, 0:7], ps[:, :, 0:7], gedge[:, si, 0], MUL)
                nc.vector.tensor_tensor(
                    ps[:, :, W - 7:W], ps[:, :, W - 7:W], gedge[:, si, 1], MUL)

            from contextlib import nullcontext
            rep_ctx = tc.For_i(0, reps) if reps > 1 else nullcontext()
            with rep_ctx:
              nc.sync.dma_start(
                  x_t[:], xs_ap.rearrange("i c (b p) w -> p i c b w", p=128))
              for img in range(IMGS_PER_CORE):
                # ---- stage 1 prep: s = sum x_c, q = sum x_c^2 ----
                x = [x_t[:, img, c] for c in range(C)]
                t0 = scr.tile([128, NB, W], F32, tag="scr")
                nc.vector.tensor_tensor(t0[:], x[0], x[1], ADD)
                nc.vector.tensor_tensor(ctr(s_t), t0[:], x[2], ADD)
                sq0 = scr.tile([128, NB, W], F32, tag="scr")
                nc.scalar.activation(sq0[:], x[0], SQUARE)
                sq1 = scr.tile([128, NB, W], F32, tag="scr")
                nc.scalar.activation(sq1[:], x[1], SQUARE)
                sq2 = scr.tile([128, NB, W], F32, tag="scr")
                nc.scalar.activation(sq2[:], x[2], SQUARE)
                t1 = scr.tile([128, NB, W], F32, tag="scr")
                nc.vector.tensor_tensor(t1[:], sq0[:], sq1[:], ADD)
                nc.vector.tensor_tensor(ctr(q_t), t1[:], sq2[:], ADD)
                for si, (k, eps) in enumerate(STAGES):
                    # box(s), box(q) with normalization folded in
                    hs, pad = hscan(s_t, k)
                    S = vband(hs, pad, si, True, "S")   # ~mean after edgefix
                    edgefix(S, si)
                    hq, _ = hscan(q_t, k)
                    Q = vband(hq, pad, si, True, "Q")
                    edgefix(Q, si)

                    m2 = scr.tile([128, NB, W], F32, tag="scr")
                    nc.scalar.activation(m2[:], S[:], SQUARE)
                    den = scr.tile([128, NB, W], F32, tag="scr")
                    nc.vector.affine_then_add(
                        den[:].rearrange("p b w -> p (b w)"),
                        m2[:].rearrange("p b w -> p (b w)"),
                        Q[:].rearrange("p b w -> p (b w)"),
                        scale=-1.0, bias=eps)
                    r = scr.tile([128, NB, W], F32, tag="scr")
                    nc.vector.reciprocal_approx_fast(
                        r[:].rearrange("p b w -> p (b w)"),
                        den[:].rearrange("p b w -> p (b w)"))
                    # A3 = 3 - 3*eps*r ; rm3 = (3*eps*r)*mean
                    nc.vector.tensor_scalar(
                        ctr(a_t), r[:], -3.0 * eps, 3.0, MUL, ADD)
                    nc.vector.scalar_tensor_tensor(
                        ctr(rm_t), r[:], 3.0 * eps, S[:], MUL, MUL)

                    ha, _ = hscan(a_t, k)
                    mA = vband(ha, pad, si, False, "mA")
                    hr, _ = hscan(rm_t, k)
                    mB = vband(hr, pad, si, False, "mB")

                    if si == 0:
                        nc.scalar.activation(G1[:, img], mA[:], COPY)
                        nc.scalar.activation(H1[:, img], mB[:], COPY)
                    elif si == 1:
                        nc.vector.tensor_tensor(G2[:, img], mA[:], G1[:, img], MUL)
                        th = scr.tile([128, NB, W], F32, tag="scr")
                        nc.vector.tensor_tensor(th[:], mA[:], H1[:, img], MUL)
                        nc.vector.tensor_tensor(Hh2[:, img], th[:], mB[:], ADD)
                        nc.vector.tensor_scalar(
                            u1[:, img], G1[:, img], -1.0, 1.0, MUL, ADD)
                        nc.vector.tensor_tensor(
                            u2[:, img], G1[:, img], G2[:, img], SUB)
                    else:
                        w3 = scr.tile([128, NB, W], F32, tag="scr")
                        nc.vector.tensor_scalar(w3[:], mA[:], -1.0, 1.0, MUL, ADD)
                        nc.vector.tensor_tensor(u3[:, img], w3[:], G2[:, img], MUL)
                        th3 = scr.tile([128, NB, W], F32, tag="scr")
                        nc.vector.tensor_tensor(th3[:], w3[:], Hh2[:, img], MUL)
                        nc.vector.tensor_tensor(v3[:, img], th3[:], mB[:], SUB)

                    if si < 2:
                        # recurrence: s' = mA*s + 3mB ; q' = mA^2 q + 2 mA mB s + 3 mB^2
                        t1r = scr.tile([128, NB, W], F32, tag="scr")
                        nc.vector.tensor_tensor(t1r[:], mA[:], ctr(s_t), MUL)
                        am = scr.tile([128, NB, W], F32, tag="scr")
                        nc.scalar.activation(am[:], mA[:], SQUARE)
                        t2r = scr.tile([128, NB, W], F32, tag="scr")
                        nc.vector.tensor_tensor(t2r[:], am[:], ctr(q_t), MUL)
                        e = scr.tile([128, NB, W], F32, tag="scr")
                        nc.vector.tensor_tensor(e[:], mB[:], t1r[:], MUL)
                        q2 = scr.tile([128, NB, W], F32, tag="scr")
                        nc.vector.scalar_tensor_tensor(
                            q2[:], e[:], 2.0, t2r[:], MUL, ADD)
                        b2 = scr.tile([128, NB, W], F32, tag="scr")
                        nc.scalar.activation(b2[:], mB[:], SQUARE)
                        # order matters: write s' after t1r, q' after t2r
                        nc.vector.scalar_tensor_tensor(
                            ctr(s_t), mB[:], 3.0, t1r[:], MUL, ADD)
                        nc.vector.scalar_tensor_tensor(
                            ctr(q_t), b2[:], 3.0, q2[:], MUL, ADD)

              # ---- fused final 1x1 conv (both images) ----
              # w1 baked as immediates: 4x tensor_scalar terms + 2x TT adds
              # on DVE; u_j*x_c products on GpSimd; V-inits on ACT.
              # V_o = (b2-b1)*H1 - b2*H2 + b3*v3   (v2 folded away)
              ob = obp.tile([128, IMGS_PER_CORE, 3, NB, W], BF16, tag="ob")
              obf = ob[:].rearrange("p i o b w -> p (i o) b w")

              def acc_term(o, src, wgt):
                  t = tp.tile([128, IMGS_PER_CORE, NB, W], BF16, tag="t")
                  nc.vector.tensor_scalar_mul(t[:], src, float(wgt))
                  nc.vector.tensor_tensor(ob[:, :, o], ob[:, :, o], t[:], ADD)

              for o in range(3):
                  nc.scalar.activation(
                      ob[:, :, o], H1[:], COPY,
                      scale=float(beta[o, 1] - beta[o, 0]))
                  acc_term(o, Hh2[:], -beta[o, 1])
                  acc_term(o, v3[:], beta[o, 2])
              for i in range(9):
                  j, c = divmod(i, 3)
                  uj = (u1, u2, u3)[j]
                  p = pp.tile([128, IMGS_PER_CORE, NB, W], BF16, tag="p")
                  nc.gpsimd.tensor_tensor(p[:], uj[:], x_t[:, :, c], MUL)
                  for o in range(3):
                      acc_term(o, p[:], w1f[o, i])
              for img in range(IMGS_PER_CORE):
                  nc.sync.dma_start(
                      out_ap[img].rearrange("o (b p) w -> p o b w", p=128),
                      ob[:, img])

    nc.compile()
    return nc


_PROGRAM_CACHE = {}


def _make_inmaps(x_hr, w1):
    import ml_dtypes
    bands, gedges = _make_consts()
    wsc = _make_wscalars(np.asarray(w1, np.float64))
    xs = np.ascontiguousarray(
        x_hr.reshape(N_CORES, IMGS_PER_CORE, C, H, W)).astype(ml_dtypes.bfloat16)
    return [
        {"xs": xs[i], "bands": bands, "gedge": gedges, "wsc": wsc}
        for i in range(N_CORES)
    ]


def kernel(x_hr: np.ndarray, w1: np.ndarray) -> np.ndarray:
    from concourse import bass_utils

    assert x_hr.shape == (16, 3, 512, 512)
    key = np.asarray(w1, np.float32).tobytes()
    nc = _PROGRAM_CACHE.get("nc") if _PROGRAM_CACHE.get("key") == key else None
    if nc is None:
        nc = _build_program(w1=w1)
        _PROGRAM_CACHE["nc"] = nc
        _PROGRAM_CACHE["key"] = key

    in_maps = _make_inmaps(np.asarray(x_hr, np.float32), w1)
    res = bass_utils.run_bass_kernel_spmd(nc, in_maps, core_ids=list(range(N_CORES)))
    out = np.stack([np.asarray(res.results[i]["out"]) for i in range(N_CORES)])
    return out.reshape(16, 3, 512, 512).astype(np.float32)



# revision 15
# speedup vs baseline: 1.2578x; 1.2578x over previous
"""DDFConvGuidedFilter Trainium2 kernel.

Data-parallel over batch: 16 images -> 8 cores x 2 images.

Per image (512x512, C=3), three guided-filter stages (k=3,7,15; eps=.16,.04,.01):
  s = sum_c x_c ; q = sum_c x_c^2
  per stage:  mean = box(s)/Nc ; Q = box(q)/Nc        (Nc = 3*cnt_h*cnt_w)
              var = Q - mean^2 ; r = 1/(var+eps)
              A3 = 3 - 3*eps*r (= 3A);  rm3 = 3*eps*r*mean (= 3b)
              mA = box(A3) ; mB = box(rm3)            (= C*box(A), C*box(b))
              next: s' = mA*s + 3mB ; q' = mA^2 q + 2 mA mB s + 3 mB^2
  F_j = G_j*x + H_j with G1=mA1, H1=mB1, G2=mA2*G1, H2=mA2*H1+mB2, ...
  out_o = sum_jc w1[o,3(j-1)+c] * (u_j * x_c)  + beta_o2*v2 + beta_o3*v3 - beta_o1*H1
    u1=1-G1, u2=G1-G2, u3=(1-mA3)*G2, v2=H1-H2, v3=(1-mA3)*H2-mB3

Box filters: horizontal pass = ONE flat DVE tensor_tensor_scan per map (the
zero margins between row-blocks self-correct the running window); vertical
pass = PE matmul against banded 128x128 Toeplitz blocks (with 1/Nc
row-normalization folded into the s/q bands; w-edge correction applied to 7
edge columns afterwards).  Final 1x1 conv = DVE scalar chains (per-partition
scalar pointers carry the w1 weights), not PE.
"""

import numpy as np

H = W = 512
C = 3
NB = 4  # h blocks of 128
OFF = 16  # left zero margin in padded tiles
BW = OFF + W + 8  # padded tile width = 536
STAGES = [(3, 0.16), (7, 0.04), (15, 0.01)]
N_CORES = 8
IMGS_PER_CORE = 2
NWS = 40  # w-scalar slots


def _cnt(k, n=512):
    i = np.arange(n)
    p = (k - 1) // 2
    return np.minimum(i + p + 1, np.minimum(2 * p + 1, n - i + p)).astype(np.float64)


def _band_block(k, dj, row_scale=None):
    """lhsT[kk, m] = scale(h_out) * 1(|128*dj + kk - m| <= pad)."""
    pad = (k - 1) // 2
    m = np.arange(128)
    kk = np.arange(128)[:, None]
    mat = (np.abs(128 * dj + kk - m) <= pad).astype(np.float64)
    if row_scale is not None:
        mat = mat * row_scale[None, :]
    return mat


def _make_consts():
    """Band matrices [24,128,128] and edge-gamma tiles [3,2,128,4,7]."""
    bands = []
    gedges = np.zeros((3, 2, 128, NB, 7), np.float64)
    for si, (k, eps) in enumerate(STAGES):
        ch = _cnt(k)
        alpha = 1.0 / (3.0 * ch * k)  # row scale: 1/(3*cnt_h(h)*k)
        # sq bands: diag0, diagM, diag3, upM, dnM
        bands.append(_band_block(k, 0, alpha[0:128]))
        bands.append(_band_block(k, 0, alpha[128:256]))
        bands.append(_band_block(k, 0, alpha[384:512]))
        bands.append(_band_block(k, 1, alpha[128:256]))  # interior rows
        bands.append(_band_block(k, -1, alpha[128:256]))
        # A/rm bands: unnormalized
        bands.append(_band_block(k, 0))
        bands.append(_band_block(k, 1))
        bands.append(_band_block(k, -1))
        cw = _cnt(k)
        gl = k / cw[0:7]
        gr = k / cw[505:512]
        gedges[si, 0] = np.broadcast_to(gl[None, None, :], (128, NB, 7))
        gedges[si, 1] = np.broadcast_to(gr[None, None, :], (128, NB, 7))
    import ml_dtypes
    return np.stack(bands).astype(ml_dtypes.bfloat16), gedges.astype(np.float32)


# band index helpers: per stage si, base = si*8
SQ_DIAG = {0: 0, 1: 1, 2: 1, 3: 2}  # per out-block
SQ_UP, SQ_DN = 3, 4
A_DIAG, A_UP, A_DN = 5, 6, 7


def _make_wscalars(w1):
    """[128, NWS] f32: w1[o,i] at o*9+i; -beta1[o], beta2[o], beta3[o] at 27+3o+j."""
    vals = np.zeros(NWS, np.float64)
    for o in range(3):
        for i in range(9):
            vals[o * 9 + i] = w1[o, i]
    beta = w1.reshape(3, 3, 3).sum(axis=2)  # [o, j]
    for o in range(3):
        vals[27 + 3 * o + 0] = -beta[o, 0]
        vals[27 + 3 * o + 1] = beta[o, 1]
        vals[27 + 3 * o + 2] = beta[o, 2]
    return np.broadcast_to(vals[None, :], (128, NWS)).astype(np.float32).copy()


def _build_program(reps=1, w1=None):
    import concourse.bacc as bacc
    import concourse.tile as tile
    from concourse import mybir

    assert w1 is not None, "w1 weights are baked as immediates"
    w1f = np.asarray(w1, np.float64)
    beta = w1f.reshape(3, 3, 3).sum(axis=2)  # [o, j]

    F32 = mybir.dt.float32
    BF16 = mybir.dt.bfloat16
    ADD = mybir.AluOpType.add
    SUB = mybir.AluOpType.subtract
    MUL = mybir.AluOpType.mult
    SQUARE = mybir.ActivationFunctionType.Square
    COPY = mybir.ActivationFunctionType.Copy

    nc = bacc.Bacc("TRN2", target_bir_lowering=False, debug=False,
                   num_devices=N_CORES)
    xs_ap = nc.dram_tensor("xs", [IMGS_PER_CORE, C, H, W], BF16,
                           kind="ExternalInput").ap()
    bands_ap = nc.dram_tensor("bands", [24, 128, 128], BF16,
                              kind="ExternalInput").ap()
    gedge_ap = nc.dram_tensor("gedge", [3, 2, 128, NB, 7], F32,
                              kind="ExternalInput").ap()
    wsc_ap = nc.dram_tensor("wsc", [128, NWS], F32, kind="ExternalInput").ap()
    out_ap = nc.dram_tensor("out", [IMGS_PER_CORE, C, H, W], BF16,
                            kind="ExternalOutput").ap()

    with tile.TileContext(nc) as tc:
        with (
            tc.tile_pool(name="const", bufs=1) as constp,
            tc.tile_pool(name="fixed", bufs=1) as fixed,
            tc.tile_pool(name="scan", bufs=1) as scanp,
            tc.tile_pool(name="scr", bufs=4) as scr,
            tc.tile_pool(name="pers", bufs=1) as pers,
            tc.tile_pool(name="pp", bufs=2) as pp,
            tc.tile_pool(name="tp", bufs=2) as tp,
            tc.tile_pool(name="obp", bufs=1) as obp,
            tc.tile_pool(name="psum", bufs=2, space="PSUM") as psum,
        ):
            # ---- constants ----
            bands = constp.tile([128, 24, 128], BF16)
            nc.sync.dma_start(bands[:], bands_ap.rearrange("n k m -> k n m"))
            gedge = constp.tile([128, 3, 2, NB, 7], F32)
            nc.sync.dma_start(gedge[:], gedge_ap.rearrange("s e p b j -> p s e b j"))
            wsc = constp.tile([128, NWS], F32)
            nc.sync.dma_start(wsc[:], wsc_ap)

            # ---- fixed padded map tiles (shared across stages & images) ----
            s_t = fixed.tile([128, NB, BW], F32, tag="s_t")
            q_t = fixed.tile([128, NB, BW], F32, tag="q_t")
            a_t = fixed.tile([128, NB, BW], F32, tag="a_t")
            rm_t = fixed.tile([128, NB, BW], F32, tag="rm_t")
            for t in (s_t, q_t, a_t, rm_t):
                nc.vector.memset(t[:], 0.0)

            # fused bf16 persists: [128, img, NB, W]
            G1 = pers.tile([128, IMGS_PER_CORE, NB, W], BF16, tag="G1")
            H1 = pers.tile([128, IMGS_PER_CORE, NB, W], BF16, tag="H1")
            G2 = pers.tile([128, IMGS_PER_CORE, NB, W], BF16, tag="G2")
            Hh2 = pers.tile([128, IMGS_PER_CORE, NB, W], BF16, tag="H2")
            u1 = pers.tile([128, IMGS_PER_CORE, NB, W], BF16, tag="u1")
            u2 = pers.tile([128, IMGS_PER_CORE, NB, W], BF16, tag="u2")
            u3, v3 = G1, G2  # slots reused after G1/G2 die
            x_t = pers.tile([128, IMGS_PER_CORE, C, NB, W], BF16, tag="x")

            def ctr(t):  # central (data) region of a padded tile
                return t[:, :, OFF:OFF + W]

            def hscan(padded, k):
                """H box pass: ONE flat scan.  hs center = [:, b, OFF+pad : OFF+pad+W]."""
                pad = (k - 1) // 2
                hs = scanp.tile([128, NB, BW], BF16, tag="hs")
                flat_in = padded[:].rearrange("p b w -> p (b w)")
                flat_out = hs[:].rearrange("p b w -> p (b w)")
                TOT = NB * BW
                nc.vector.tensor_tensor_scan(
                    flat_out[:, k:TOT],
                    flat_in[:, k:TOT],
                    flat_in[:, 0:TOT - k],
                    0.0, ADD, SUB,
                )
                return hs, pad

            def vband(hs, pad, si, sq, tag):
                """V box pass on PE -> psum tile [128, NB, 512]."""
                ps = psum.tile([128, NB, W], F32, tag="ps")
                base = si * 8
                o0 = OFF + pad
                for b in range(NB):
                    js = [j for j in (b - 1, b, b + 1) if 0 <= j < NB]
                    for idx, j in enumerate(js):
                        if sq:
                            bi = base + (SQ_DIAG[b] if j == b else
                                         (SQ_UP if j == b + 1 else SQ_DN))
                        else:
                            bi = base + (A_DIAG if j == b else
                                         (A_UP if j == b + 1 else A_DN))
                        nc.tensor.matmul(
                            ps[:, b, :], bands[:, bi, :],
                            hs[:, j, o0:o0 + W],
                            start=(idx == 0), stop=(idx == len(js) - 1),
                        )
                return ps

            def edgefix(ps, si):
                """multiply 7 left/right edge columns by gamma (in-place, PSUM)."""
                nc.vector.tensor_tensor(
                    ps[:, :, 0:7], ps[:, :, 0:7], gedge[:, si, 0], MUL)
                nc.vector.tensor_tensor(
                    ps[:, :, W - 7:W], ps[:, :, W - 7:W], gedge[:, si, 1], MUL)

            from contextlib import nullcontext
            # amortize the per-iteration loop barrier over 2 pipeline reps
            unroll = 2 if reps > 1 and reps % 2 == 0 else 1
            rep_ctx = (tc.For_i(0, reps // unroll) if reps > 1
                       else nullcontext())
            with rep_ctx:
             for _u in range(unroll):
              nc.sync.dma_start(
                  x_t[:], xs_ap.rearrange("i c (b p) w -> p i c b w", p=128))
              for img in range(IMGS_PER_CORE):
                # ---- stage 1 prep: s = sum x_c, q = sum x_c^2 ----
                x = [x_t[:, img, c] for c in range(C)]
                t0 = scr.tile([128, NB, W], F32, tag="scr")
                nc.vector.tensor_tensor(t0[:], x[0], x[1], ADD)
                nc.vector.tensor_tensor(ctr(s_t), t0[:], x[2], ADD)
                sq0 = scr.tile([128, NB, W], F32, tag="scr")
                nc.scalar.activation(sq0[:], x[0], SQUARE)
                sq1 = scr.tile([128, NB, W], F32, tag="scr")
                nc.scalar.activation(sq1[:], x[1], SQUARE)
                sq2 = scr.tile([128, NB, W], F32, tag="scr")
                nc.scalar.activation(sq2[:], x[2], SQUARE)
                t1 = scr.tile([128, NB, W], F32, tag="scr")
                nc.vector.tensor_tensor(t1[:], sq0[:], sq1[:], ADD)
                nc.vector.tensor_tensor(ctr(q_t), t1[:], sq2[:], ADD)
                for si, (k, eps) in enumerate(STAGES):
                    # box(s), box(q) with normalization folded in
                    hs, pad = hscan(s_t, k)
                    S = vband(hs, pad, si, True, "S")   # ~mean after edgefix
                    edgefix(S, si)
                    hq, _ = hscan(q_t, k)
                    Q = vband(hq, pad, si, True, "Q")
                    edgefix(Q, si)

                    m2 = scr.tile([128, NB, W], F32, tag="scr")
                    nc.scalar.activation(m2[:], S[:], SQUARE)
                    den = scr.tile([128, NB, W], F32, tag="scr")
                    nc.vector.affine_then_add(
                        den[:].rearrange("p b w -> p (b w)"),
                        m2[:].rearrange("p b w -> p (b w)"),
                        Q[:].rearrange("p b w -> p (b w)"),
                        scale=-1.0, bias=eps)
                    r = scr.tile([128, NB, W], F32, tag="scr")
                    nc.vector.reciprocal_approx_fast(
                        r[:].rearrange("p b w -> p (b w)"),
                        den[:].rearrange("p b w -> p (b w)"))
                    # A3 = 3 - 3*eps*r ; rm3 = (3*eps*r)*mean
                    nc.vector.tensor_scalar(
                        ctr(a_t), r[:], -3.0 * eps, 3.0, MUL, ADD)
                    nc.vector.scalar_tensor_tensor(
                        ctr(rm_t), r[:], 3.0 * eps, S[:], MUL, MUL)

                    ha, _ = hscan(a_t, k)
                    mA = vband(ha, pad, si, False, "mA")
                    hr, _ = hscan(rm_t, k)
                    mB = vband(hr, pad, si, False, "mB")

                    if si == 0:
                        nc.scalar.activation(G1[:, img], mA[:], COPY)
                        nc.scalar.activation(H1[:, img], mB[:], COPY)
                    elif si == 1:
                        nc.vector.tensor_tensor(G2[:, img], mA[:], G1[:, img], MUL)
                        th = scr.tile([128, NB, W], F32, tag="scr")
                        nc.vector.tensor_tensor(th[:], mA[:], H1[:, img], MUL)
                        nc.vector.tensor_tensor(Hh2[:, img], th[:], mB[:], ADD)
                        nc.vector.tensor_scalar(
                            u1[:, img], G1[:, img], -1.0, 1.0, MUL, ADD)
                        nc.vector.tensor_tensor(
                            u2[:, img], G1[:, img], G2[:, img], SUB)
                    else:
                        w3 = scr.tile([128, NB, W], F32, tag="scr")
                        nc.vector.tensor_scalar(w3[:], mA[:], -1.0, 1.0, MUL, ADD)
                        nc.vector.tensor_tensor(u3[:, img], w3[:], G2[:, img], MUL)
                        th3 = scr.tile([128, NB, W], F32, tag="scr")
                        nc.vector.tensor_tensor(th3[:], w3[:], Hh2[:, img], MUL)
                        nc.vector.tensor_tensor(v3[:, img], th3[:], mB[:], SUB)

                    if si < 2:
                        # recurrence: s' = mA*s + 3mB ; q' = mA^2 q + 2 mA mB s + 3 mB^2
                        t1r = scr.tile([128, NB, W], F32, tag="scr")
                        nc.vector.tensor_tensor(t1r[:], mA[:], ctr(s_t), MUL)
                        am = scr.tile([128, NB, W], F32, tag="scr")
                        nc.scalar.activation(am[:], mA[:], SQUARE)
                        t2r = scr.tile([128, NB, W], F32, tag="scr")
                        nc.vector.tensor_tensor(t2r[:], am[:], ctr(q_t), MUL)
                        e = scr.tile([128, NB, W], F32, tag="scr")
                        nc.vector.tensor_tensor(e[:], mB[:], t1r[:], MUL)
                        q2 = scr.tile([128, NB, W], F32, tag="scr")
                        nc.vector.scalar_tensor_tensor(
                            q2[:], e[:], 2.0, t2r[:], MUL, ADD)
                        b2 = scr.tile([128, NB, W], F32, tag="scr")
                        nc.scalar.activation(b2[:], mB[:], SQUARE)
                        # order matters: write s' after t1r, q' after t2r
                        nc.vector.scalar_tensor_tensor(
                            ctr(s_t), mB[:], 3.0, t1r[:], MUL, ADD)
                        nc.vector.scalar_tensor_tensor(
                            ctr(q_t), b2[:], 3.0, q2[:], MUL, ADD)

              # ---- fused final 1x1 conv (both images) ----
              # w1 baked as immediates: 4x tensor_scalar terms + 2x TT adds
              # on DVE; u_j*x_c products on GpSimd; V-inits on ACT.
              # V_o = (b2-b1)*H1 - b2*H2 + b3*v3   (v2 folded away)
              ob = obp.tile([128, IMGS_PER_CORE, 3, NB, W], BF16, tag="ob")
              obf = ob[:].rearrange("p i o b w -> p (i o) b w")

              def acc_term(o, src, wgt):
                  t = tp.tile([128, IMGS_PER_CORE, NB, W], BF16, tag="t")
                  nc.vector.tensor_scalar_mul(t[:], src, float(wgt))
                  nc.vector.tensor_tensor(ob[:, :, o], ob[:, :, o], t[:], ADD)

              for o in range(3):
                  nc.scalar.activation(
                      ob[:, :, o], H1[:], COPY,
                      scale=float(beta[o, 1] - beta[o, 0]))
                  acc_term(o, Hh2[:], -beta[o, 1])
                  acc_term(o, v3[:], beta[o, 2])
              for i in range(9):
                  j, c = divmod(i, 3)
                  uj = (u1, u2, u3)[j]
                  p = pp.tile([128, IMGS_PER_CORE, NB, W], BF16, tag="p")
                  nc.gpsimd.tensor_tensor(p[:], uj[:], x_t[:, :, c], MUL)
                  for o in range(3):
                      acc_term(o, p[:], w1f[o, i])
              for img in range(IMGS_PER_CORE):
                  nc.sync.dma_start(
                      out_ap[img].rearrange("o (b p) w -> p o b w", p=128),
                      ob[:, img])

    nc.compile()
    return nc


_PROGRAM_CACHE = {}


def _make_inmaps(x_hr, w1):
    import ml_dtypes
    bands, gedges = _make_consts()
    wsc = _make_wscalars(np.asarray(w1, np.float64))
    xs = np.ascontiguousarray(
        x_hr.reshape(N_CORES, IMGS_PER_CORE, C, H, W)).astype(ml_dtypes.bfloat16)
    return [
        {"xs": xs[i], "bands": bands, "gedge": gedges, "wsc": wsc}
        for i in range(N_CORES)
    ]


def kernel(x_hr: np.ndarray, w1: np.ndarray) -> np.ndarray:
    from concourse import bass_utils

    assert x_hr.shape == (16, 3, 512, 512)
    key = np.asarray(w1, np.float32).tobytes()
    nc = _PROGRAM_CACHE.get("nc") if _PROGRAM_CACHE.get("key") == key else None
    if nc is None:
        nc = _build_program(w1=w1)
        _PROGRAM_CACHE["nc"] = nc
        _PROGRAM_CACHE["key"] = key

    in_maps = _make_inmaps(np.asarray(x_hr, np.float32), w1)
    res = bass_utils.run_bass_kernel_spmd(nc, in_maps, core_ids=list(range(N_CORES)))
    out = np.stack([np.asarray(res.results[i]["out"]) for i in range(N_CORES)])
    return out.reshape(16, 3, 512, 512).astype(np.float32)



# revision 16
# speedup vs baseline: 1.2751x; 1.0138x over previous
"""DDFConvGuidedFilter Trainium2 kernel.

Data-parallel over batch: 16 images -> 8 cores x 2 images.

Per image (512x512, C=3), three guided-filter stages (k=3,7,15; eps=.16,.04,.01):
  s = sum_c x_c ; q = sum_c x_c^2
  per stage:  mean = box(s)/Nc ; Q = box(q)/Nc        (Nc = 3*cnt_h*cnt_w)
              var = Q - mean^2 ; r = 1/(var+eps)
              A3 = 3 - 3*eps*r (= 3A);  rm3 = 3*eps*r*mean (= 3b)
              mA = box(A3) ; mB = box(rm3)            (= C*box(A), C*box(b))
              next: s' = mA*s + 3mB ; q' = mA^2 q + 2 mA mB s + 3 mB^2
  F_j = G_j*x + H_j with G1=mA1, H1=mB1, G2=mA2*G1, H2=mA2*H1+mB2, ...
  out_o = sum_jc w1[o,3(j-1)+c] * (u_j * x_c)  + beta_o2*v2 + beta_o3*v3 - beta_o1*H1
    u1=1-G1, u2=G1-G2, u3=(1-mA3)*G2, v2=H1-H2, v3=(1-mA3)*H2-mB3

Box filters: horizontal pass = ONE flat DVE tensor_tensor_scan per map (the
zero margins between row-blocks self-correct the running window); vertical
pass = PE matmul against banded 128x128 Toeplitz blocks (with 1/Nc
row-normalization folded into the s/q bands; w-edge correction applied to 7
edge columns afterwards).  Final 1x1 conv = DVE scalar chains (per-partition
scalar pointers carry the w1 weights), not PE.
"""

import numpy as np

H = W = 512
C = 3
NB = 4  # h blocks of 128
OFF = 16  # left zero margin in padded tiles
BW = OFF + W + 8  # padded tile width = 536
STAGES = [(3, 0.16), (7, 0.04), (15, 0.01)]
N_CORES = 8
IMGS_PER_CORE = 2
NWS = 40  # w-scalar slots


def _cnt(k, n=512):
    i = np.arange(n)
    p = (k - 1) // 2
    return np.minimum(i + p + 1, np.minimum(2 * p + 1, n - i + p)).astype(np.float64)


def _band_block(k, dj, row_scale=None):
    """lhsT[kk, m] = scale(h_out) * 1(|128*dj + kk - m| <= pad)."""
    pad = (k - 1) // 2
    m = np.arange(128)
    kk = np.arange(128)[:, None]
    mat = (np.abs(128 * dj + kk - m) <= pad).astype(np.float64)
    if row_scale is not None:
        mat = mat * row_scale[None, :]
    return mat


def _make_consts():
    """Band matrices [24,128,128] and edge-gamma tiles [3,2,128,4,7]."""
    bands = []
    gedges = np.zeros((3, 2, 128, NB, 7), np.float64)
    for si, (k, eps) in enumerate(STAGES):
        ch = _cnt(k)
        alpha = 1.0 / (3.0 * ch * k)  # row scale: 1/(3*cnt_h(h)*k)
        # sq bands: diag0, diagM, diag3, upM, dnM
        bands.append(_band_block(k, 0, alpha[0:128]))
        bands.append(_band_block(k, 0, alpha[128:256]))
        bands.append(_band_block(k, 0, alpha[384:512]))
        bands.append(_band_block(k, 1, alpha[128:256]))  # interior rows
        bands.append(_band_block(k, -1, alpha[128:256]))
        # A/rm bands: unnormalized
        bands.append(_band_block(k, 0))
        bands.append(_band_block(k, 1))
        bands.append(_band_block(k, -1))
        cw = _cnt(k)
        gl = k / cw[0:7]
        gr = k / cw[505:512]
        gedges[si, 0] = np.broadcast_to(gl[None, None, :], (128, NB, 7))
        gedges[si, 1] = np.broadcast_to(gr[None, None, :], (128, NB, 7))
    import ml_dtypes
    return np.stack(bands).astype(ml_dtypes.bfloat16), gedges.astype(np.float32)


# band index helpers: per stage si, base = si*8
SQ_DIAG = {0: 0, 1: 1, 2: 1, 3: 2}  # per out-block
SQ_UP, SQ_DN = 3, 4
A_DIAG, A_UP, A_DN = 5, 6, 7


def _make_wscalars(w1):
    """[128, NWS] f32: w1[o,i] at o*9+i; -beta1[o], beta2[o], beta3[o] at 27+3o+j."""
    vals = np.zeros(NWS, np.float64)
    for o in range(3):
        for i in range(9):
            vals[o * 9 + i] = w1[o, i]
    beta = w1.reshape(3, 3, 3).sum(axis=2)  # [o, j]
    for o in range(3):
        vals[27 + 3 * o + 0] = -beta[o, 0]
        vals[27 + 3 * o + 1] = beta[o, 1]
        vals[27 + 3 * o + 2] = beta[o, 2]
    return np.broadcast_to(vals[None, :], (128, NWS)).astype(np.float32).copy()


def _build_program(reps=1, w1=None):
    import concourse.bacc as bacc
    import concourse.tile as tile
    from concourse import mybir

    assert w1 is not None, "w1 weights are baked as immediates"
    w1f = np.asarray(w1, np.float64)
    beta = w1f.reshape(3, 3, 3).sum(axis=2)  # [o, j]

    F32 = mybir.dt.float32
    BF16 = mybir.dt.bfloat16
    ADD = mybir.AluOpType.add
    SUB = mybir.AluOpType.subtract
    MUL = mybir.AluOpType.mult
    SQUARE = mybir.ActivationFunctionType.Square
    COPY = mybir.ActivationFunctionType.Copy

    nc = bacc.Bacc("TRN2", target_bir_lowering=False, debug=False,
                   num_devices=N_CORES)
    xs_ap = nc.dram_tensor("xs", [IMGS_PER_CORE, C, H, W], BF16,
                           kind="ExternalInput").ap()
    bands_ap = nc.dram_tensor("bands", [24, 128, 128], BF16,
                              kind="ExternalInput").ap()
    gedge_ap = nc.dram_tensor("gedge", [3, 2, 128, NB, 7], F32,
                              kind="ExternalInput").ap()
    wsc_ap = nc.dram_tensor("wsc", [128, NWS], F32, kind="ExternalInput").ap()
    out_ap = nc.dram_tensor("out", [IMGS_PER_CORE, C, H, W], BF16,
                            kind="ExternalOutput").ap()

    with tile.TileContext(nc) as tc:
        with (
            tc.tile_pool(name="const", bufs=1) as constp,
            tc.tile_pool(name="fixed", bufs=1) as fixed,
            tc.tile_pool(name="scan", bufs=1) as scanp,
            tc.tile_pool(name="scr", bufs=4) as scr,
            tc.tile_pool(name="pers", bufs=1) as pers,
            tc.tile_pool(name="pp", bufs=2) as pp,
            tc.tile_pool(name="tp", bufs=2) as tp,
            tc.tile_pool(name="obp", bufs=1) as obp,
            tc.tile_pool(name="psum", bufs=2, space="PSUM") as psum,
        ):
            # ---- constants ----
            bands = constp.tile([128, 24, 128], BF16)
            nc.sync.dma_start(bands[:], bands_ap.rearrange("n k m -> k n m"))
            gedge = constp.tile([128, 3, 2, NB, 7], F32)
            nc.sync.dma_start(gedge[:], gedge_ap.rearrange("s e p b j -> p s e b j"))
            wsc = constp.tile([128, NWS], F32)
            nc.sync.dma_start(wsc[:], wsc_ap)

            # ---- fixed padded map tiles (shared across stages & images) ----
            s_t = fixed.tile([128, NB, BW], F32, tag="s_t")
            q_t = fixed.tile([128, NB, BW], F32, tag="q_t")
            a_t = fixed.tile([128, NB, BW], F32, tag="a_t")
            rm_t = fixed.tile([128, NB, BW], F32, tag="rm_t")
            for t in (s_t, q_t, a_t, rm_t):
                nc.vector.memset(t[:], 0.0)

            # fused bf16 persists: [128, img, NB, W]
            G1 = pers.tile([128, IMGS_PER_CORE, NB, W], BF16, tag="G1")
            H1 = pers.tile([128, IMGS_PER_CORE, NB, W], BF16, tag="H1")
            G2 = pers.tile([128, IMGS_PER_CORE, NB, W], BF16, tag="G2")
            Hh2 = pers.tile([128, IMGS_PER_CORE, NB, W], BF16, tag="H2")
            u1 = pers.tile([128, IMGS_PER_CORE, NB, W], BF16, tag="u1")
            u2 = pers.tile([128, IMGS_PER_CORE, NB, W], BF16, tag="u2")
            u3, v3 = G1, G2  # slots reused after G1/G2 die
            x_t = pers.tile([128, IMGS_PER_CORE, C, NB, W], BF16, tag="x")

            def ctr(t):  # central (data) region of a padded tile
                return t[:, :, OFF:OFF + W]

            def hscan(padded, k):
                """H box pass: ONE flat scan.  hs center = [:, b, OFF+pad : OFF+pad+W]."""
                pad = (k - 1) // 2
                hs = scanp.tile([128, NB, BW], BF16, tag="hs")
                flat_in = padded[:].rearrange("p b w -> p (b w)")
                flat_out = hs[:].rearrange("p b w -> p (b w)")
                TOT = NB * BW
                nc.vector.tensor_tensor_scan(
                    flat_out[:, k:TOT],
                    flat_in[:, k:TOT],
                    flat_in[:, 0:TOT - k],
                    0.0, ADD, SUB,
                )
                return hs, pad

            def vband(hs, pad, si, sq, tag):
                """V box pass on PE -> psum tile [128, NB, 512]."""
                ps = psum.tile([128, NB, W], F32, tag="ps")
                base = si * 8
                o0 = OFF + pad
                for b in range(NB):
                    js = [j for j in (b - 1, b, b + 1) if 0 <= j < NB]
                    for idx, j in enumerate(js):
                        if sq:
                            bi = base + (SQ_DIAG[b] if j == b else
                                         (SQ_UP if j == b + 1 else SQ_DN))
                        else:
                            bi = base + (A_DIAG if j == b else
                                         (A_UP if j == b + 1 else A_DN))
                        nc.tensor.matmul(
                            ps[:, b, :], bands[:, bi, :],
                            hs[:, j, o0:o0 + W],
                            start=(idx == 0), stop=(idx == len(js) - 1),
                        )
                return ps

            def edgefix(ps, si):
                """multiply 7 left/right edge columns by gamma (in-place, PSUM)."""
                nc.vector.tensor_tensor(
                    ps[:, :, 0:7], ps[:, :, 0:7], gedge[:, si, 0], MUL)
                nc.vector.tensor_tensor(
                    ps[:, :, W - 7:W], ps[:, :, W - 7:W], gedge[:, si, 1], MUL)

            from contextlib import nullcontext
            # amortize the per-iteration loop barrier over several reps
            unroll = next((u for u in (4, 2, 1)
                           if reps > 1 and reps % u == 0), 1)
            rep_ctx = (tc.For_i(0, reps // unroll) if reps > 1
                       else nullcontext())
            with rep_ctx:
             for _u in range(unroll):
              nc.sync.dma_start(
                  x_t[:], xs_ap.rearrange("i c (b p) w -> p i c b w", p=128))
              for img in range(IMGS_PER_CORE):
                # ---- stage 1 prep: s = sum x_c, q = sum x_c^2 ----
                x = [x_t[:, img, c] for c in range(C)]
                t0 = scr.tile([128, NB, W], F32, tag="scr")
                nc.vector.tensor_tensor(t0[:], x[0], x[1], ADD)
                nc.vector.tensor_tensor(ctr(s_t), t0[:], x[2], ADD)
                sq0 = scr.tile([128, NB, W], F32, tag="scr")
                nc.scalar.activation(sq0[:], x[0], SQUARE)
                sq1 = scr.tile([128, NB, W], F32, tag="scr")
                nc.scalar.activation(sq1[:], x[1], SQUARE)
                sq2 = scr.tile([128, NB, W], F32, tag="scr")
                nc.scalar.activation(sq2[:], x[2], SQUARE)
                t1 = scr.tile([128, NB, W], F32, tag="scr")
                nc.vector.tensor_tensor(t1[:], sq0[:], sq1[:], ADD)
                nc.vector.tensor_tensor(ctr(q_t), t1[:], sq2[:], ADD)
                for si, (k, eps) in enumerate(STAGES):
                    # box(s), box(q) with normalization folded in
                    hs, pad = hscan(s_t, k)
                    S = vband(hs, pad, si, True, "S")   # ~mean after edgefix
                    edgefix(S, si)
                    hq, _ = hscan(q_t, k)
                    Q = vband(hq, pad, si, True, "Q")
                    edgefix(Q, si)

                    m2 = scr.tile([128, NB, W], F32, tag="scr")
                    nc.scalar.activation(m2[:], S[:], SQUARE)
                    den = scr.tile([128, NB, W], F32, tag="scr")
                    nc.vector.affine_then_add(
                        den[:].rearrange("p b w -> p (b w)"),
                        m2[:].rearrange("p b w -> p (b w)"),
                        Q[:].rearrange("p b w -> p (b w)"),
                        scale=-1.0, bias=eps)
                    r = scr.tile([128, NB, W], F32, tag="scr")
                    nc.vector.reciprocal_approx_fast(
                        r[:].rearrange("p b w -> p (b w)"),
                        den[:].rearrange("p b w -> p (b w)"))
                    # A3 = 3 - 3*eps*r ; rm3 = (3*eps*r)*mean
                    nc.vector.tensor_scalar(
                        ctr(a_t), r[:], -3.0 * eps, 3.0, MUL, ADD)
                    nc.vector.scalar_tensor_tensor(
                        ctr(rm_t), r[:], 3.0 * eps, S[:], MUL, MUL)

                    ha, _ = hscan(a_t, k)
                    mA = vband(ha, pad, si, False, "mA")
                    hr, _ = hscan(rm_t, k)
                    mB = vband(hr, pad, si, False, "mB")

                    if si == 0:
                        nc.scalar.activation(G1[:, img], mA[:], COPY)
                        nc.scalar.activation(H1[:, img], mB[:], COPY)
                    elif si == 1:
                        nc.vector.tensor_tensor(G2[:, img], mA[:], G1[:, img], MUL)
                        th = scr.tile([128, NB, W], F32, tag="scr")
                        nc.vector.tensor_tensor(th[:], mA[:], H1[:, img], MUL)
                        nc.vector.tensor_tensor(Hh2[:, img], th[:], mB[:], ADD)
                        nc.vector.tensor_scalar(
                            u1[:, img], G1[:, img], -1.0, 1.0, MUL, ADD)
                        nc.vector.tensor_tensor(
                            u2[:, img], G1[:, img], G2[:, img], SUB)
                    else:
                        w3 = scr.tile([128, NB, W], F32, tag="scr")
                        nc.vector.tensor_scalar(w3[:], mA[:], -1.0, 1.0, MUL, ADD)
                        nc.vector.tensor_tensor(u3[:, img], w3[:], G2[:, img], MUL)
                        th3 = scr.tile([128, NB, W], F32, tag="scr")
                        nc.vector.tensor_tensor(th3[:], w3[:], Hh2[:, img], MUL)
                        nc.vector.tensor_tensor(v3[:, img], th3[:], mB[:], SUB)

                    if si < 2:
                        # recurrence: s' = mA*s + 3mB ; q' = mA^2 q + 2 mA mB s + 3 mB^2
                        t1r = scr.tile([128, NB, W], F32, tag="scr")
                        nc.vector.tensor_tensor(t1r[:], mA[:], ctr(s_t), MUL)
                        am = scr.tile([128, NB, W], F32, tag="scr")
                        nc.scalar.activation(am[:], mA[:], SQUARE)
                        t2r = scr.tile([128, NB, W], F32, tag="scr")
                        nc.vector.tensor_tensor(t2r[:], am[:], ctr(q_t), MUL)
                        e = scr.tile([128, NB, W], F32, tag="scr")
                        nc.vector.tensor_tensor(e[:], mB[:], t1r[:], MUL)
                        q2 = scr.tile([128, NB, W], F32, tag="scr")
                        nc.vector.scalar_tensor_tensor(
                            q2[:], e[:], 2.0, t2r[:], MUL, ADD)
                        b2 = scr.tile([128, NB, W], F32, tag="scr")
                        nc.scalar.activation(b2[:], mB[:], SQUARE)
                        # order matters: write s' after t1r, q' after t2r
                        nc.vector.scalar_tensor_tensor(
                            ctr(s_t), mB[:], 3.0, t1r[:], MUL, ADD)
                        nc.vector.scalar_tensor_tensor(
                            ctr(q_t), b2[:], 3.0, q2[:], MUL, ADD)

              # ---- fused final 1x1 conv (both images) ----
              # w1 baked as immediates: 4x tensor_scalar terms + 2x TT adds
              # on DVE; u_j*x_c products on GpSimd; V-inits on ACT.
              # V_o = (b2-b1)*H1 - b2*H2 + b3*v3   (v2 folded away)
              ob = obp.tile([128, IMGS_PER_CORE, 3, NB, W], BF16, tag="ob")
              obf = ob[:].rearrange("p i o b w -> p (i o) b w")

              def acc_term(o, src, wgt):
                  t = tp.tile([128, IMGS_PER_CORE, NB, W], BF16, tag="t")
                  nc.vector.tensor_scalar_mul(t[:], src, float(wgt))
                  nc.vector.tensor_tensor(ob[:, :, o], ob[:, :, o], t[:], ADD)

              for o in range(3):
                  nc.scalar.activation(
                      ob[:, :, o], H1[:], COPY,
                      scale=float(beta[o, 1] - beta[o, 0]))
                  acc_term(o, Hh2[:], -beta[o, 1])
                  acc_term(o, v3[:], beta[o, 2])
              for i in range(9):
                  j, c = divmod(i, 3)
                  uj = (u1, u2, u3)[j]
                  p = pp.tile([128, IMGS_PER_CORE, NB, W], BF16, tag="p")
                  nc.gpsimd.tensor_tensor(p[:], uj[:], x_t[:, :, c], MUL)
                  for o in range(3):
                      acc_term(o, p[:], w1f[o, i])
              for img in range(IMGS_PER_CORE):
                  nc.sync.dma_start(
                      out_ap[img].rearrange("o (b p) w -> p o b w", p=128),
                      ob[:, img])

    nc.compile()
    return nc


_PROGRAM_CACHE = {}


def _make_inmaps(x_hr, w1):
    import ml_dtypes
    bands, gedges = _make_consts()
    wsc = _make_wscalars(np.asarray(w1, np.float64))
    xs = np.ascontiguousarray(
        x_hr.reshape(N_CORES, IMGS_PER_CORE, C, H, W)).astype(ml_dtypes.bfloat16)
    return [
        {"xs": xs[i], "bands": bands, "gedge": gedges, "wsc": wsc}
        for i in range(N_CORES)
    ]


def kernel(x_hr: np.ndarray, w1: np.ndarray) -> np.ndarray:
    from concourse import bass_utils

    assert x_hr.shape == (16, 3, 512, 512)
    key = np.asarray(w1, np.float32).tobytes()
    nc = _PROGRAM_CACHE.get("nc") if _PROGRAM_CACHE.get("key") == key else None
    if nc is None:
        nc = _build_program(w1=w1)
        _PROGRAM_CACHE["nc"] = nc
        _PROGRAM_CACHE["key"] = key

    in_maps = _make_inmaps(np.asarray(x_hr, np.float32), w1)
    res = bass_utils.run_bass_kernel_spmd(nc, in_maps, core_ids=list(range(N_CORES)))
    out = np.stack([np.asarray(res.results[i]["out"]) for i in range(N_CORES)])
    return out.reshape(16, 3, 512, 512).astype(np.float32)

